# revision 1
# baseline (speedup 1.0000x reference)
"""Trainium2 Bass kernel for nn_ConvolutionalAttention_3015067042131.

Math (reference.py):
  x [16,128,64,64] f32; x1 = x[:, :64], x2 = x[:, 64:]
  pooled = mean(x1, HW); h = gelu(pooled @ w1.T + b1); dyn = (h @ w2.T + b2) -> [B,64,9]
  x1_dyn = per-(batch,channel) 3x3 depthwise conv of x1 with dyn
  x1_lk  = conv2d(x1, lk_filter[64,64,13,13], SAME)
  out = concat([x1_lk + x1_dyn, x2], ch)

Strategy:
  * The tiny MLP (dyn) is computed on host in float64 (0.0007% of FLOPs).
  * The dynamic depthwise 3x3 is folded into the 13x13 conv weights as
    per-batch diagonal additions on the central 3x3 taps (3x3 tap (u,v)
    == 13x13 tap (u+5, v+5)); so the device runs ONE dense 13x13 conv
    with per-batch weights on 6 of 91 weight tiles.
  * Conv as shift-and-matmul: for each kernel tap, out[o, pix] +=
    W_tap[c, o].T @ xpad[c, pix+off]. Taps are paired along K: SBUF
    partitions 0-63 hold the zero-padded image (76x76), partitions
    64-127 hold it shifted one column left, so taps (i,j) and (i,j+1)
    fuse into one K=128 matmul. 91 matmuls cover all 169 taps.
  * Output pixels processed in 8 chunks of 512 (8 rows). Chunk pairs run
    CONCURRENTLY in the two PE column halves via tile_position (0,0) /
    (0,64) writing PSUM partitions 0-63 / 64-127 (measured 2x).
  * fp16 operands (measured: HW fp16 matmul exact on rounded inputs,
    fp32 PSUM accumulate; end-to-end rel err ~3e-4). f32r is broken in
    this stack (device-crashing) and fp32 runs at 1/4 rate.
  * Sharding: data-parallel over batch, 2 batches per core on 8 cores.
    x2 passthrough is host-side (no device work).
"""
import math

import numpy as np

B, C, H, W = 16, 128, 64, 64
PDIM, SK, LK = 64, 3, 13
PAD = LK // 2  # 6
HP, WP = H + 2 * PAD, W + 2 * PAD  # 76, 76
NCORES = 8
BPC = B // NCORES  # batches per core
NP = 91            # weight tiles (78 tap pairs + 13 singles)
NCHUNK = 8         # 512-pixel chunks per image
CHUNK = H * W // NCHUNK  # 512

# tile t = i*7 + p: p in 0..5 -> pair ((i,2p),(i,2p+1)); p == 6 -> single (i,12)
_TAP_A = {}
for _i in range(LK):
    for _p in range(7):
        _TAP_A[_i * 7 + _p] = (_i, 2 * _p if _p < 6 else 12)

# central 3x3 taps (i,j in 5..7) live in pair tiles i*7+2 (B-half j=5) and
# i*7+3 (A-half j=6, B-half j=7); those 6 tiles are per-batch.
_MOD_TILES = [5 * 7 + 2, 6 * 7 + 2, 7 * 7 + 2, 5 * 7 + 3, 6 * 7 + 3, 7 * 7 + 3]
_MOD_SLOT = {t: s for s, t in enumerate(_MOD_TILES)}

_ERF = np.vectorize(math.erf, otypes=[np.float64])

_CACHED_NC = None


def _build_nc():
    import concourse.mybir as mybir
    import concourse.tile as tile
    from concourse import bacc

    f32 = mybir.dt.float32
    f16 = mybir.dt.float16

    nc = bacc.Bacc(None, target_bir_lowering=False)
    xs = nc.dram_tensor("xs", [BPC, PDIM, H, W], f16, kind="ExternalInput")
    wsh = nc.dram_tensor("wsh", [128, NP * 64], f16, kind="ExternalInput")
    wmod = nc.dram_tensor("wmod", [BPC, 128, 6 * 64], f16, kind="ExternalInput")
    y = nc.dram_tensor("y", [BPC, PDIM, H * W], f32, kind="ExternalOutput")

    with tile.TileContext(nc) as tc:
        with (
            tc.tile_pool(name="wpool", bufs=1) as wpool,
            tc.tile_pool(name="wmpool", bufs=2) as wmpool,
            tc.tile_pool(name="xpool", bufs=2) as xpool,
            tc.tile_pool(name="opool", bufs=3) as opool,
            tc.tile_pool(name="pspool", bufs=4, space="PSUM") as pspool,
        ):
            wsh_sb = wpool.tile([128, NP * 64], f16)
            nc.sync.dma_start(out=wsh_sb[:], in_=wsh[:])

            # PE warmup: ~10 junk matmuls on a zeroed scratch tile so the
            # HAM un-throttles (1.2 -> 2.4 GHz) while the input DMAs run.
            scratch = wpool.tile([128, CHUNK], f16)
            nc.vector.memset(scratch[:], 0.0)
            ps_warm = pspool.tile([128, CHUNK], f32, name="ps_warm", bufs=1)
            for wi in range(26):
                nc.tensor.matmul(
                    ps_warm[0:64, :],
                    lhsT=scratch[:, 0:64],
                    rhs=scratch[:, :],
                    start=(wi == 0),
                    stop=(wi == 25),
                    skip_group_check=True,
                )

            for b in range(BPC):
                wm = wmpool.tile([128, 6 * 64], f16)
                nc.sync.dma_start(out=wm[:], in_=wmod[b, :, :])
                # Contiguous DMA (8KB runs/partition, fast) of the image into
                # BOTH partition halves of a staging tile, on two queues; the
                # strided padded layout is then built on-chip by DVE (the
                # direct strided DMA measured ~10x slower).
                xst = xpool.tile([128, H, W], f16, name="xst")
                nc.sync.dma_start(out=xst[0:64, :, :], in_=xs[b, :, :, :])
                nc.sync.dma_start(out=xst[64:128, :, :], in_=xs[b, :, :, :])
                xp = xpool.tile([128, HP, WP], f16)
                # border-only memsets, disjoint from the copied interiors so
                # nothing serializes behind them
                nc.vector.memset(xp[:, 0:PAD, :], 0.0)              # top rows
                nc.vector.memset(xp[:, PAD + H :, :], 0.0)          # bottom rows
                nc.vector.memset(xp[0:64, PAD : PAD + H, 0:PAD], 0.0)
                nc.vector.memset(xp[0:64, PAD : PAD + H, PAD + W :], 0.0)
                nc.vector.memset(xp[64:128, PAD : PAD + H, 0 : PAD - 1], 0.0)
                nc.vector.memset(xp[64:128, PAD : PAD + H, PAD - 1 + W :], 0.0)
                # partitions 0-63: padded image; 64-127: shifted left 1 col
                nc.vector.tensor_copy(
                    xp[0:64, PAD : PAD + H, PAD : PAD + W], xst[0:64, :, :]
                )
                # scalar engine so both halves reshape concurrently
                nc.scalar.copy(
                    xp[64:128, PAD : PAD + H, PAD - 1 : PAD - 1 + W],
                    xst[64:128, :, :],
                )
                for cp in range(NCHUNK // 2):
                    ps = pspool.tile([128, CHUNK], f32)
                    for t in range(NP):
                        s = _MOD_SLOT.get(t)
                        w_ap = (
                            wm[:, s * 64 : (s + 1) * 64]
                            if s is not None
                            else wsh_sb[:, t * 64 : (t + 1) * 64]
                        )
                        i, j = _TAP_A[t]
                        for half in (0, 1):
                            r0 = i + 8 * (2 * cp + half)
                            nc.tensor.matmul(
                                ps[64 * half : 64 * (half + 1), :],
                                lhsT=w_ap,
                                rhs=xp[:, r0 : r0 + 8, j : j + 64],
                                start=(t == 0),
                                stop=(t == NP - 1),
                                tile_position=(0, 64 * half),
                                skip_group_check=True,
                            )
                    ot = opool.tile([128, CHUNK], f32)
                    nc.vector.tensor_copy(ot[:], ps[:])
                    nc.sync.dma_start(
                        out=y[b, :, (2 * cp) * CHUNK : (2 * cp + 1) * CHUNK],
                        in_=ot[0:64, :],
                    )
                    nc.sync.dma_start(
                        out=y[b, :, (2 * cp + 1) * CHUNK : (2 * cp + 2) * CHUNK],
                        in_=ot[64:128, :],
                    )
    nc.compile()
    return nc


def _get_nc():
    global _CACHED_NC
    if _CACHED_NC is None:
        _CACHED_NC = _build_nc()
    return _CACHED_NC


def _host_dyn(x, w1, b1, w2, b2):
    """dwc_proj MLP on host, float64: dyn [B, 64, 9]."""
    pooled = x[:, :PDIM].mean(axis=(2, 3), dtype=np.float64)      # [B, 64]
    z = pooled @ w1.T.astype(np.float64) + b1.astype(np.float64)  # [B, 32]
    h = 0.5 * z * (1.0 + _ERF(z / math.sqrt(2.0)))                # exact gelu
    dyn = h @ w2.T.astype(np.float64) + b2.astype(np.float64)     # [B, 576]
    return dyn.reshape(B, PDIM, SK * SK)


def _host_weights(lk_filter, dyn):
    """Build shared tap-pair weight tiles + per-batch modified central tiles.

    Weight tile t [128, 64]: rows 0-63 = lk[o, c, iA, jA].T (tap A), rows
    64-127 = tap B = (iA, jA+1), zeros for singles. lhsT layout [K=c, M=o].
    """
    lkT = lk_filter.transpose(1, 0, 2, 3).astype(np.float32)  # [c, o, i, j]
    Wt = np.zeros((NP, 128, 64), np.float32)
    for t, (i, jA) in _TAP_A.items():
        Wt[t, 0:64, :] = lkT[:, :, i, jA]
        if jA < 12:
            Wt[t, 64:128, :] = lkT[:, :, i, jA + 1]

    ar = np.arange(64)
    Wmod = np.zeros((B, 6, 128, 64), np.float32)
    for ii, i in enumerate((5, 6, 7)):
        t2, t3 = i * 7 + 2, i * 7 + 3
        u = i - 5
        for b in range(B):
            m2 = Wt[t2].copy()
            m3 = Wt[t3].copy()
            m2[64 + ar, ar] += dyn[b, :, u * 3 + 0].astype(np.float32)  # tap (i,5)
            m3[ar, ar] += dyn[b, :, u * 3 + 1].astype(np.float32)       # tap (i,6)
            m3[64 + ar, ar] += dyn[b, :, u * 3 + 2].astype(np.float32)  # tap (i,7)
            Wmod[b, ii] = m2
            Wmod[b, 3 + ii] = m3

    wsh_np = np.ascontiguousarray(
        Wt.transpose(1, 0, 2).reshape(128, NP * 64)
    ).astype(np.float16)
    wmod_np = np.ascontiguousarray(
        Wmod.transpose(0, 2, 1, 3).reshape(B, 128, 6 * 64)
    ).astype(np.float16)
    return wsh_np, wmod_np


def kernel(x, lk_filter, w1, b1, w2, b2):
    from concourse.bass_utils import run_bass_kernel_spmd

    x = np.asarray(x, dtype=np.float32)
    dyn = _host_dyn(x, np.asarray(w1), np.asarray(b1), np.asarray(w2), np.asarray(b2))
    wsh_np, wmod_np = _host_weights(np.asarray(lk_filter, dtype=np.float32), dyn)

    x1_f16 = x[:, :PDIM].astype(np.float16)  # [16, 64, 64, 64]

    nc = _get_nc()
    in_maps = []
    for k in range(NCORES):
        b0 = k * BPC
        in_maps.append(
            {
                "xs": np.ascontiguousarray(x1_f16[b0 : b0 + BPC]),
                "wsh": wsh_np,
                "wmod": np.ascontiguousarray(wmod_np[b0 : b0 + BPC]),
            }
        )
    res = run_bass_kernel_spmd(nc, in_maps, core_ids=list(range(NCORES)))

    out = np.empty((B, C, H, W), np.float32)
    for k in range(NCORES):
        b0 = k * BPC
        out[b0 : b0 + BPC, :PDIM] = res.results[k]["y"].reshape(BPC, PDIM, H, W)
    out[:, PDIM:] = x[:, PDIM:]
    return out



# revision 3
# speedup vs baseline: 1.1277x; 1.1277x over previous
"""Trainium2 Bass kernel for nn_ConvolutionalAttention_3015067042131.

Math (reference.py):
  x [16,128,64,64] f32; x1 = x[:, :64], x2 = x[:, 64:]
  pooled = mean(x1, HW); h = gelu(pooled @ w1.T + b1); dyn = (h @ w2.T + b2) -> [B,64,9]
  x1_dyn = per-(batch,channel) 3x3 depthwise conv of x1 with dyn
  x1_lk  = conv2d(x1, lk_filter[64,64,13,13], SAME)
  out = concat([x1_lk + x1_dyn, x2], ch)

Strategy:
  * The tiny MLP (dyn) is computed on host in float64 (0.0007% of FLOPs).
  * The dynamic depthwise 3x3 is folded into the 13x13 conv weights as
    per-batch diagonal additions on the central 3x3 taps (3x3 tap (u,v)
    == 13x13 tap (u+5, v+5)); the device runs ONE dense 13x13 conv.
  * Conv as shift-and-matmul with K-dim tap pairing: SBUF partitions
    0-63 hold the zero-padded image, 64-127 hold it shifted one column
    left, so taps (i,2p) and (i,2p+1) fuse into one K=128 matmul
    (78 tiles).  The 13 leftover j=12 taps are paired ROW-wise via a
    second small buffer xpr whose partitions 64-127 hold the padded
    image shifted UP one row: taps (2k,12)+(2k+1,12) fuse (6 tiles),
    (12,12) rides alone (1 tile).  85 tiles total vs the optimal
    ceil(169/2)=85 -> 99.4% K-packing efficiency.
  * Output pixels processed in 8 chunks of 512 (8 rows). Chunk pairs run
    CONCURRENTLY in the two PE column halves via tile_position (0,0) /
    (0,64) writing PSUM partitions 0-63 / 64-127 (measured 2x).
  * fp16 operands (measured end-to-end rel err ~3e-4; f32r is broken in
    this stack and fp32 runs at 1/4 rate). Output stored as f16
    (adds ~<5e-4 elementwise rounding, halves output DMA bytes).
  * Startup critical path (was 24.7us to first real matmul): image DMAs
    are issued BEFORE the weights and split row-wise; weight DMAs are
    issued on the scalar engine's hardware DGE queue so they don't
    serialize behind the image issues on sync; the shared-weight DMA is
    split in 3 so early tap tiles land first; padded-layout interior
    copies run on vector (the scalar-engine copy was 3x slower),
    border memsets on gpsimd/vector.
  * Sharding: data-parallel over batch, 2 batches per core on 8 cores.
    x2 passthrough is host-side (no device work).
"""
import math

import numpy as np

B, C, H, W = 16, 128, 64, 64
PDIM, SK, LK = 64, 3, 13
PAD = LK // 2  # 6
HP, WP = H + 2 * PAD, W + 2 * PAD  # 76, 76
NCORES = 8
BPC = B // NCORES  # batches per core
NP = 85            # 78 col-pair tiles + 6 row-pair j=12 tiles + 1 single
NCHUNK = 8         # 512-pixel chunks per image
CHUNK = H * W // NCHUNK  # 512

# pair tile t = i*6 + p (t<78): taps (i,2p) half-A / (i,2p+1) half-B.
# c12 tile t = 78+k (k<6): taps (2k,12) half-A / (2k+1,12) half-B (xpr).
# t = 84: tap (12,12) half-A only (half-B weight zero).

# central 3x3 taps (i,j in 5..7): j=5 -> half-B of tile i*6+2,
# j=6/7 -> halves A/B of tile i*6+3; those 6 tiles are per-batch.
_MOD_TILES = [5 * 6 + 2, 6 * 6 + 2, 7 * 6 + 2, 5 * 6 + 3, 6 * 6 + 3, 7 * 6 + 3]
_MOD_SLOT = {t: s for s, t in enumerate(_MOD_TILES)}

# wsh DMA split points (tiles): early tap tiles must land first
_WCHUNKS = [(0, 8), (8, 40), (40, NP)]

_ERF = np.vectorize(math.erf, otypes=[np.float64])

_CACHED_NC = None


def _build_nc():
    import concourse.mybir as mybir
    import concourse.tile as tile
    from concourse import bacc

    f32 = mybir.dt.float32
    f16 = mybir.dt.float16

    nc = bacc.Bacc(None, target_bir_lowering=False)
    xs = nc.dram_tensor("xs", [BPC, PDIM, H, W], f16, kind="ExternalInput")
    wsh = nc.dram_tensor("wsh", [128, NP * 64], f16, kind="ExternalInput")
    wmod = nc.dram_tensor("wmod", [BPC, 128, 6 * 64], f16, kind="ExternalInput")
    # chunk-major output: row ci*64+ch holds output rows 8ci..8ci+7 of ch
    y = nc.dram_tensor("y", [BPC, NCHUNK * 64, CHUNK], f16, kind="ExternalOutput")

    with tile.TileContext(nc) as tc:
        with (
            tc.tile_pool(name="wpool", bufs=1) as wpool,
            tc.tile_pool(name="wmpool", bufs=2) as wmpool,
            tc.tile_pool(name="xstpool", bufs=2) as xstpool,
            tc.tile_pool(name="xppool", bufs=2) as xppool,
            tc.tile_pool(name="xprpool", bufs=2) as xprpool,
            tc.tile_pool(name="opool", bufs=3) as opool,
            tc.tile_pool(name="pspool", bufs=4, space="PSUM") as pspool,
        ):
            # ---- warmup scratch (gpsimd memset so vector stays free) ----
            scratch = wpool.tile([128, CHUNK], f16)
            nc.gpsimd.memset(scratch[:], 0.0)

            wsh_sb = wpool.tile([128, NP * 64], f16)
            wm = [wmpool.tile([128, 6 * 64], f16, name=f"wm{b}") for b in range(BPC)]
            xst = [xstpool.tile([128, H, W], f16, name=f"xst{b}") for b in range(BPC)]
            xp = [xppool.tile([128, HP, WP], f16, name=f"xp{b}") for b in range(BPC)]
            xpr = [
                xprpool.tile([128, HP, 64], f16, name=f"xpr{b}") for b in range(BPC)
            ]

            # ---- DMA issue. Images on sync, weights on scalar (both are
            # hardware-DGE queues) so neither serializes behind the other.
            # b0 image, row-split so the padded-layout build can pipeline.
            nc.sync.dma_start(out=xst[0][0:64, 0:32, :], in_=xs[0, :, 0:32, :])
            nc.sync.dma_start(out=xst[0][64:128, 0:32, :], in_=xs[0, :, 0:32, :])
            nc.scalar.dma_start(
                out=wsh_sb[:, _WCHUNKS[0][0] * 64 : _WCHUNKS[0][1] * 64],
                in_=wsh[:, _WCHUNKS[0][0] * 64 : _WCHUNKS[0][1] * 64],
            )
            nc.scalar.dma_start(out=wm[0][:], in_=wmod[0, :, :])
            nc.sync.dma_start(out=xst[0][0:64, 32:64, :], in_=xs[0, :, 32:64, :])
            nc.sync.dma_start(out=xst[0][64:128, 32:64, :], in_=xs[0, :, 32:64, :])
            for c0, c1 in _WCHUNKS[1:]:
                nc.scalar.dma_start(
                    out=wsh_sb[:, c0 * 64 : c1 * 64], in_=wsh[:, c0 * 64 : c1 * 64]
                )
            nc.sync.dma_start(out=xst[1][0:64, :, :], in_=xs[1, :, :, :])
            nc.sync.dma_start(out=xst[1][64:128, :, :], in_=xs[1, :, :, :])
            nc.scalar.dma_start(out=wm[1][:], in_=wmod[1, :, :])

            # ---- PE warmup: junk matmuls so the HAM clock ramps while the
            # input DMAs are in flight.
            ps_warm = pspool.tile([128, CHUNK], mybir.dt.float32, name="ps_warm", bufs=1)
            for wi in range(5):
                nc.tensor.matmul(
                    ps_warm[0:64, :],
                    lhsT=scratch[:, 0:64],
                    rhs=scratch[:, :],
                    start=(wi == 0),
                    stop=(wi == 4),
                    skip_group_check=True,
                )

            # ---- b0 xp borders on vector (fast, must precede first matmul)
            def xp_borders(eng, t):
                eng.memset(t[:, 0:PAD, :], 0.0)
                eng.memset(t[:, PAD + H :, :], 0.0)
                eng.memset(t[0:64, PAD : PAD + H, 0:PAD], 0.0)
                eng.memset(t[0:64, PAD : PAD + H, PAD + W :], 0.0)
                eng.memset(t[64:128, PAD : PAD + H, 0 : PAD - 1], 0.0)
                eng.memset(t[64:128, PAD : PAD + H, PAD - 1 + W :], 0.0)

            def xpr_borders(eng, t):
                # half-A: padded rows at col offset 12; half-B same shifted
                # up one row.  cols 58:64 map past the padded width -> zero.
                eng.memset(t[0:64, 0:PAD, :], 0.0)
                eng.memset(t[0:64, PAD + H :, :], 0.0)
                eng.memset(t[0:64, PAD : PAD + H, 58:64], 0.0)
                eng.memset(t[64:128, 0 : PAD - 1, :], 0.0)
                eng.memset(t[64:128, PAD - 1 + H :, :], 0.0)
                eng.memset(t[64:128, PAD - 1 : PAD - 1 + H, 58:64], 0.0)

            xp_borders(nc.vector, xp[0])
            # remaining borders are needed much later -> gpsimd
            xpr_borders(nc.gpsimd, xpr[0])
            xp_borders(nc.gpsimd, xp[1])
            xpr_borders(nc.gpsimd, xpr[1])

            # ---- interior copies on vector (scalar's ACTIVATE copy is 3x
            # slower). b0 row-split to pipeline with the split DMAs; the
            # first chunk-pair only reads padded rows < 38, i.e. the top
            # copies, so matmuls start before the bottom halves land.
            for b in range(BPC):
                if b == 0:
                    for r0, r1 in ((0, 32), (32, 64)):
                        nc.vector.tensor_copy(
                            xp[b][0:64, PAD + r0 : PAD + r1, PAD : PAD + W],
                            xst[b][0:64, r0:r1, :],
                        )
                        nc.vector.tensor_copy(
                            xp[b][64:128, PAD + r0 : PAD + r1, PAD - 1 : PAD - 1 + W],
                            xst[b][64:128, r0:r1, :],
                        )
                else:
                    nc.vector.tensor_copy(
                        xp[b][0:64, PAD : PAD + H, PAD : PAD + W], xst[b][0:64, :, :]
                    )
                    nc.vector.tensor_copy(
                        xp[b][64:128, PAD : PAD + H, PAD - 1 : PAD - 1 + W],
                        xst[b][64:128, :, :],
                    )
                # xpr: half-A = padded img cols 12.. (img cols 6..63);
                # half-B = same, shifted up one row.
                nc.vector.tensor_copy(
                    xpr[b][0:64, PAD : PAD + H, 0:58], xst[b][0:64, :, 6:64]
                )
                nc.vector.tensor_copy(
                    xpr[b][64:128, PAD - 1 : PAD - 1 + H, 0:58],
                    xst[b][64:128, :, 6:64],
                )

            # ---- main matmul stream ----
            for b in range(BPC):
                for cp in range(NCHUNK // 2):
                    ps = pspool.tile([128, CHUNK], mybir.dt.float32)
                    for t in range(NP):
                        s = _MOD_SLOT.get(t)
                        w_ap = (
                            wm[b][:, s * 64 : (s + 1) * 64]
                            if s is not None
                            else wsh_sb[:, t * 64 : (t + 1) * 64]
                        )
                        if t < 78:
                            row, col, src = t // 6, 2 * (t % 6), xp[b]
                        elif t < 84:
                            row, col, src = 2 * (t - 78), 0, xpr[b]
                        else:
                            row, col, src = 12, 0, xpr[b]
                        for half in (0, 1):
                            r0 = row + 8 * (2 * cp + half)
                            nc.tensor.matmul(
                                ps[64 * half : 64 * (half + 1), :],
                                lhsT=w_ap,
                                rhs=src[:, r0 : r0 + 8, col : col + 64],
                                start=(t == 0),
                                stop=(t == NP - 1),
                                tile_position=(0, 64 * half),
                                skip_group_check=True,
                            )
                    ot = opool.tile([128, CHUNK], f16)
                    nc.vector.tensor_copy(ot[:], ps[:])
                    nc.sync.dma_start(
                        out=y[b, (2 * cp) * 64 : (2 * cp + 2) * 64, :], in_=ot[:]
                    )
    nc.compile()
    return nc


def _get_nc():
    global _CACHED_NC
    if _CACHED_NC is None:
        _CACHED_NC = _build_nc()
    return _CACHED_NC


def _host_dyn(x, w1, b1, w2, b2):
    """dwc_proj MLP on host, float64: dyn [B, 64, 9]."""
    pooled = x[:, :PDIM].mean(axis=(2, 3), dtype=np.float64)      # [B, 64]
    z = pooled @ w1.T.astype(np.float64) + b1.astype(np.float64)  # [B, 32]
    h = 0.5 * z * (1.0 + _ERF(z / math.sqrt(2.0)))                # exact gelu
    dyn = h @ w2.T.astype(np.float64) + b2.astype(np.float64)     # [B, 576]
    return dyn.reshape(B, PDIM, SK * SK)


def _host_weights(lk_filter, dyn):
    """Build shared tap-pair weight tiles + per-batch modified central tiles.

    Weight tile t [128, 64]: rows 0-63 = lk[o, c, iA, jA].T (tap A), rows
    64-127 = tap B, zeros for the lone (12,12) half. lhsT layout [K=c, M=o].
    """
    lkT = lk_filter.transpose(1, 0, 2, 3).astype(np.float32)  # [c, o, i, j]
    Wt = np.zeros((NP, 128, 64), np.float32)
    for i in range(LK):
        for p in range(6):
            t = i * 6 + p
            Wt[t, 0:64, :] = lkT[:, :, i, 2 * p]
            Wt[t, 64:128, :] = lkT[:, :, i, 2 * p + 1]
    for k in range(6):
        Wt[78 + k, 0:64, :] = lkT[:, :, 2 * k, 12]
        Wt[78 + k, 64:128, :] = lkT[:, :, 2 * k + 1, 12]
    Wt[84, 0:64, :] = lkT[:, :, 12, 12]

    ar = np.arange(64)
    Wmod = np.zeros((B, 6, 128, 64), np.float32)
    for ii, i in enumerate((5, 6, 7)):
        t2, t3 = i * 6 + 2, i * 6 + 3
        u = i - 5
        for b in range(B):
            m2 = Wt[t2].copy()
            m3 = Wt[t3].copy()
            m2[64 + ar, ar] += dyn[b, :, u * 3 + 0].astype(np.float32)  # tap (i,5)
            m3[ar, ar] += dyn[b, :, u * 3 + 1].astype(np.float32)       # tap (i,6)
            m3[64 + ar, ar] += dyn[b, :, u * 3 + 2].astype(np.float32)  # tap (i,7)
            Wmod[b, ii] = m2
            Wmod[b, 3 + ii] = m3

    wsh_np = np.ascontiguousarray(
        Wt.transpose(1, 0, 2).reshape(128, NP * 64)
    ).astype(np.float16)
    wmod_np = np.ascontiguousarray(
        Wmod.transpose(0, 2, 1, 3).reshape(B, 128, 6 * 64)
    ).astype(np.float16)
    return wsh_np, wmod_np


def kernel(x, lk_filter, w1, b1, w2, b2):
    from concourse.bass_utils import run_bass_kernel_spmd

    x = np.asarray(x, dtype=np.float32)
    dyn = _host_dyn(x, np.asarray(w1), np.asarray(b1), np.asarray(w2), np.asarray(b2))
    wsh_np, wmod_np = _host_weights(np.asarray(lk_filter, dtype=np.float32), dyn)

    x1_f16 = x[:, :PDIM].astype(np.float16)  # [16, 64, 64, 64]

    nc = _get_nc()
    in_maps = []
    for k in range(NCORES):
        b0 = k * BPC
        in_maps.append(
            {
                "xs": np.ascontiguousarray(x1_f16[b0 : b0 + BPC]),
                "wsh": wsh_np,
                "wmod": np.ascontiguousarray(wmod_np[b0 : b0 + BPC]),
            }
        )
    res = run_bass_kernel_spmd(nc, in_maps, core_ids=list(range(NCORES)))

    out = np.empty((B, C, H, W), np.float32)
    for k in range(NCORES):
        b0 = k * BPC
        yk = res.results[k]["y"].astype(np.float32)          # [BPC, 512, 512]
        yk = yk.reshape(BPC, NCHUNK, 64, CHUNK).transpose(0, 2, 1, 3)
        out[b0 : b0 + BPC, :PDIM] = yk.reshape(BPC, PDIM, H, W)
    out[:, PDIM:] = x[:, PDIM:]
    return out
